# revision 30
# baseline (speedup 1.0000x reference)
"""v7: halved T1/T2 variant of the final kernel.

Nadaraya-Watson kernel regression (retrieval_knn) on 8 NeuronCores.

out[b,d] = sum_n y[n,d] * G(u[n,d]-v[b,d]) / sum_n G(...),
G(z) = exp(-z^2/2); u = mlp(calc_X)/h, v = mlp(x)/h (1/h folded into W2).

Sharding: N-parallel over the reference set (1024 rows/core); every core
sees all B=512 queries and returns partial num/den sums; the host sums
partials across cores and divides (tiny [512,16] reduce).

Per-core plan, built for MINIMAL instruction count (measured cost here is
dominated by fixed per-rep + per-instruction overheads, not engine time):
  - one packed DMA loads W1T|W2Ta|W2Tb|xT|XTs; one fused MLP over the
    1536 columns [x.T | calc_X.T-slice] (6+6 matmuls, 2 relus) gives
    vu = [v[16,512] | u[16,1024]] / h; a DRAM bounce broadcasts it to
    V[p=(16r+d), g] (fp16) and U[p, n] (fp16).
  - main pass in layout [p=(16r+d), free=(g=64, n=1024)] with stride-0
    broadcast APs, 5 giant ops (65536 elems each):
      T1 (DVE):  W = U - V            (fp16 work tile, 128KB/partition)
      T2 (ACT):  W = DerivErf(W/sqrt2) = (2/sqrt(pi)) exp(-(u-v)^2/2)
                 (constant cancels in the num/den ratio)
      R1 (DVE):  den[p, g] = sum_n W
      T3 (DVE):  W = W * Yrep  (in-place)
      R2 (DVE):  num[p, g] = sum_n W
  - ND [128, (den 64 | num 64)] fp32 -> single output DMA.
Host: sums ND over cores; out[8g+r, d] = num[16r+d, g]/den[16r+d, g].
"""
import sys
sys.path.insert(0, '/opt/trn_rl_repo')
import numpy as np
from concourse import bass, tile, bacc, mybir
from concourse.bass_utils import run_bass_kernel_spmd

F32 = mybir.dt.float32
F16 = mybir.dt.float16
AF = mybir.ActivationFunctionType
ALU = mybir.AluOpType

B, N, DIN, DMID, DOUT = 512, 8192, 128, 256, 16
NCORES = 8
NSL = N // NCORES           # 1024 reference rows per core
NG = B // 8                 # 64 query groups; b = 8g+r, p = 16r+d
CPW = DMID + 2 * DOUT + B + NSL      # packed consts width: 1824
XOFF = DMID + 2 * DOUT               # xT offset in pack: 288
ISQ2 = float(0.5 ** 0.5)


def build_kernel(reps=1, sim=False, ng=NG, nmain=5, den_eng="A", num_eng="D"):
    nc = bacc.Bacc(None, target_bir_lowering=False)

    CP_d = nc.dram_tensor("CP", [DIN, CPW], F32, kind="ExternalInput")
    YT_d = nc.dram_tensor("YTs", [DOUT, NSL], F16, kind="ExternalInput")
    nd_d = nc.dram_tensor("nd_out", [128, 2 * NG], F32, kind="ExternalOutput")

    with tile.TileContext(nc) as tc:
      for _rep in range(reps):
        with (
            tc.tile_pool(name="dram", bufs=1, space="DRAM") as dram,
            tc.tile_pool(name="sb", bufs=1) as sb,
        ):
            CP = sb.tile([DIN, CPW], F32)
            nc.sync.dma_start(CP[:], CP_d[:])
            Yrep = sb.tile([128, NSL], F16)
            nc.sync.dma_start(
                Yrep[:], bass.AP(YT_d[:].tensor, 0,
                                 [[0, 8], [NSL, DOUT], [1, NSL]]))

            # ---- fused MLP over 1536 cols [xT | XTs]: vu = [v | u] ----
            H = sb.tile([DIN, 2, B + NSL], F32)
            with tc.tile_pool(name="ps1", bufs=1, space="PSUM") as ps1:
                for j in range(2):
                    PH = ps1.tile([DIN, B + NSL], F32, tag="ph")
                    for k in range(3):
                        nc.tensor.matmul(
                            PH[:, 512 * k:512 * (k + 1)],
                            CP[:, 128 * j:128 * (j + 1)],
                            CP[:, XOFF + 512 * k:XOFF + 512 * (k + 1)])
                    nc.vector.tensor_scalar_max(H[:, j, :], PH[:], 0.0)
            VU = sb.tile([DOUT, B + NSL], F16)
            with tc.tile_pool(name="ps2", bufs=1, space="PSUM") as ps2:
                PZ = ps2.tile([DOUT, B + NSL], F32, tag="pz")
                for k in range(3):
                    for j in range(2):
                        nc.tensor.matmul(
                            PZ[:, 512 * k:512 * (k + 1)],
                            CP[:, DMID + DOUT * j:DMID + DOUT * (j + 1)],
                            H[:, j, 512 * k:512 * (k + 1)],
                            start=(j == 0), stop=(j == 1))
                nc.vector.tensor_copy(VU[:], PZ[:])
            vu_dram = dram.tile([DOUT, B + NSL], F16)
            nc.sync.dma_start(vu_dram[:], VU[:])
            # V[16r+d, g] = v[d, 8g+r];  U[16r+d, n] = u[d, n]
            # vq_dram[16r+d, g] = v[d, 8g+r], stored pre-arranged
            vq_dram = dram.tile([128, NG], F16)
            nc.sync.dma_start(
                bass.AP(vq_dram[:].tensor, 0,
                        [[NG, DOUT], [1, NG], [NG * DOUT, 8]]),
                VU[:, 0:B].rearrange("d (g r) -> d g r", g=NG))
            V = sb.tile([128, NG], F16)
            nc.sync.dma_start(V[:], vq_dram[:])
            U = sb.tile([128, NSL], F16)
            nc.sync.dma_start(
                U[:], bass.AP(vu_dram[:].tensor, B,
                              [[0, 8], [B + NSL, DOUT], [1, NSL]]))

            # ---- main pass: 5 giant ops over [128, 64, 1024] ----
            ND = sb.tile([128, 2, NG], F32)
            # inner dim padded by 8 so [g, n] can't flatten to one 65536-count
            # AP dim (16-bit ISA num_elem field caps at 65535)
            Wt = sb.tile([128, NG, NSL + 8], F16)
            W = Wt[:, :, 0:NSL]
            Ub = U[:].rearrange("p (o n) -> p o n", o=1).broadcast_to(
                [128, ng, NSL])
            Vb = V[:, 0:ng].rearrange("p (g o) -> p g o", o=1).broadcast_to(
                [128, ng, NSL])
            Yb = Yrep[:].rearrange("p (o n) -> p o n", o=1).broadcast_to(
                [128, ng, NSL])
            ngh = max(ng // 2, 1)
            Ubh = U[:].rearrange("p (o n) -> p o n", o=1).broadcast_to(
                [128, ngh, NSL])
            for h in range(ng // ngh):
                sl = slice(ngh * h, ngh * (h + 1))
                Vbh = V[:, sl].rearrange(
                    "p (g o) -> p g o", o=1).broadcast_to([128, ngh, NSL])
                Wh = Wt[:, sl, 0:NSL]
                nc.vector.tensor_tensor(Wh, Ubh, Vbh, op=ALU.subtract)
                nc.scalar.activation(Wh, Wh, AF.Derivative_Erf, scale=ISQ2)
            if nmain >= 3:
                # per-group accumulation passes (TensorReduce is ~8x slower
                # per element than these op classes here): den via in-place
                # ACT Copy + accum, num via in-place DVE STT(mult Y) + accum;
                # the two engines pipeline across g.
                for g in range(ng):
                    Wg = Wt[:, g, 0:NSL]
                    de = den_eng[g % len(den_eng)]
                    if de == "A":
                        nc.scalar.activation(Wg, Wg, AF.Copy,
                                             accum_out=ND[:, 0, g:g + 1])
                    elif de == "S":
                        nc.vector.scalar_tensor_tensor(
                            Wg, Wg, 1.0, Yrep[:], op0=ALU.mult,
                            op1=ALU.bypass, accum_out=ND[:, 0, g:g + 1])
                    else:
                        eng = nc.gpsimd if de == "P" else nc.vector
                        eng.tensor_scalar(Wg, Wg, 1.0, 0.0, op0=ALU.mult,
                                          op1=ALU.add,
                                          accum_out=ND[:, 0, g:g + 1])
                    ne = num_eng[g % len(num_eng)]
                    eng = nc.gpsimd if ne == "P" else nc.vector
                    eng.scalar_tensor_tensor(
                        Wg, Wg, 1.0, Yrep[:], op0=ALU.bypass, op1=ALU.mult,
                        accum_out=ND[:, 1, g:g + 1])
            if nmain < 3:
                # debug-timing variants: keep ND written so the out DMA works
                nc.vector.tensor_copy(ND[:, 0, :], V[:])
                nc.vector.tensor_copy(ND[:, 1, :], V[:])
            nc.sync.dma_start(nd_d[:], ND[:])

    nc.compile()
    return nc


_NC = None


def prep_in_maps(inputs):
    x = np.asarray(inputs["x"], dtype=np.float32)
    calc_X = np.asarray(inputs["calc_X"], dtype=np.float32)
    calc_Y = np.asarray(inputs["calc_Y"], dtype=np.float32)
    W1 = np.asarray(inputs["W1"], dtype=np.float32)
    W2 = np.asarray(inputs["W2"], dtype=np.float32)
    h = float(np.asarray(inputs["h"], dtype=np.float32).reshape(-1)[0])

    XT = np.ascontiguousarray(calc_X.T)                 # [128, 8192]
    xT = np.ascontiguousarray(x.T)                      # [128, 512]
    W1T = np.ascontiguousarray(W1.T)                    # [128, 256]
    W2Th = np.ascontiguousarray(W2.T) / h               # [256, 16], 1/h folded
    YTf = calc_Y.T.astype(np.float16)                   # [16, 8192]

    in_maps = []
    for c in range(NCORES):
        CP = np.concatenate(
            [W1T, W2Th[0:128], W2Th[128:256], xT,
             XT[:, NSL * c:NSL * (c + 1)]], axis=1)
        in_maps.append({
            "CP": np.ascontiguousarray(CP),
            "YTs": np.ascontiguousarray(YTf[:, NSL * c:NSL * (c + 1)]),
        })
    return in_maps


def combine_results(core_outs):
    """core_outs: list of [128, 2*NG] partials -> [B, DOUT] output."""
    nd = np.sum([np.asarray(o, dtype=np.float64) for o in core_outs], axis=0)
    nd = nd.reshape(8, DOUT, 2, NG)                     # [r, d, (den|num), g]
    den = nd[:, :, 0, :]
    num = nd[:, :, 1, :]
    out = num / den                                     # [r, d, g]
    return np.ascontiguousarray(
        out.transpose(2, 0, 1).reshape(B, DOUT)).astype(np.float32)


def kernel(**inputs):
    global _NC
    in_maps = prep_in_maps(inputs)
    if _NC is None:
        _NC = build_kernel()
    res = run_bass_kernel_spmd(_NC, in_maps, core_ids=list(range(NCORES)))
    return combine_results([res.results[c]["nd_out"] for c in range(NCORES)])


if __name__ == "__main__":
    rng = np.random.default_rng(0)
    ins = {
        "x": rng.standard_normal((B, DIN), dtype=np.float32),
        "calc_X": rng.standard_normal((N, DIN), dtype=np.float32),
        "calc_Y": rng.standard_normal((N, DOUT), dtype=np.float32),
        "W1": (rng.standard_normal((DMID, DIN), dtype=np.float32) * DIN ** -0.5),
        "W2": (rng.standard_normal((DOUT, DMID), dtype=np.float32) * DMID ** -0.5),
        "h": np.array([1.5], dtype=np.float32),
    }
    out = kernel(**ins)
    def mlp(v):
        return np.maximum(v @ ins["W1"].T, 0.0) @ ins["W2"].T
    Zw = mlp(ins["x"]); Xw = mlp(ins["calc_X"])
    z = (Xw[None] - Zw[:, None]) / ins["h"][0]
    w = np.exp(-0.5 * z * z)
    ref = (w * ins["calc_Y"][None]).sum(1) / w.sum(1)
    rel = np.abs(out - ref).max() / np.abs(ref).max()
    print("rel err:", rel)
